# revision 19
# baseline (speedup 1.0000x reference)
"""Trainium2 Bass kernel for nn_CCSequenceModel (2-layer GRU encoder + autoregressive
2-layer GRU decoder with gated output head).

Strategy: pure data parallel over 8 NeuronCores (batch 8192 -> 1024/core).
On-chip layout: gate/hidden dim on partitions, batch on the free dim, processed
as two 512-sample halves (PSUM bank limit). All SBUF tensors bf16, PSUM fp32.

Per GRU cell (input dim D, hidden 64), with the cell's h living in a fixed
64-partition block ("blend side") of its stream tile and r/hn on the other block:
  P1 = W1^T @ S   (128,M) psum  -> cols arranged [z | r] so z lands blend-side
  P2 = W2^T @ S   (128,M) psum  -> [in | hn], in on blend side, hn on r side
  rz = sigmoid(P1 + b_rz)                       (ACT, per-partition bias)
  rhn = (P2[r-side] + b_hn) * rz[r-side]        (DVE scalar_tensor_tensor)
  P2[blend] += I^T @ rhn                        (PE identity accumulation)
  n = tanh(P2[blend] + b_in)                    (ACT)
  u = h - n ; u = z*u ; h' = n + u              (DVE tensor_tensor)
Biases ride in ACT bias / STT scalar operands; weights are pre-packed host-side.
x is pre-transposed host-side to (L, 6, B_core) bf16 and DMA'd per step into the
layer-0 stream tile. Decoder output cv is DMA'd per step to a DRAM staging
buffer (T, B_core); the host does the final (t,b)->(b,t) transpose.
"""
import sys
import numpy as np

for _p in ('/opt/trn_rl_repo', '/root/.axon_site/_ro/trn_rl_repo'):
    if _p not in sys.path:
        sys.path.insert(0, _p)

import ml_dtypes
import concourse.bass as bass
import concourse.tile as tile
from concourse import bacc, mybir
from concourse.bass_utils import run_bass_kernel_spmd

BF16 = mybir.dt.bfloat16
F32 = mybir.dt.float32
NPBF = ml_dtypes.bfloat16
ALU = mybir.AluOpType
ACTF = mybir.ActivationFunctionType

H = 64
NIN = 6
NCORES = 8
T_OUT = 180  # decoder length (fixed by the model)

_BUILD_CACHE = {}


# ------------------------------------------------------------------ host prep
def _pack_cell(Wih, Whh, bih, bhh, in_rows, h_rows, blend_lo, K):
    """Pack one GRU cell's weights into stationary matrices + bias vectors.

    in_rows/h_rows: slices of the stream-tile partition range carrying the
    cell input x and hidden h. blend_lo: True if the cell's blend block is
    partitions 0:64 (z/in on cols 0:64, r/hn on cols 64:128).
    Returns W1 (K,128), W2 (K,128) float32, b1 (128,), b2 (128,).
    """
    Wih = np.asarray(Wih, np.float32)
    Whh = np.asarray(Whh, np.float32)
    bih = np.asarray(bih, np.float32)
    bhh = np.asarray(bhh, np.float32)
    W1 = np.zeros((K, 128), np.float32)
    W2 = np.zeros((K, 128), np.float32)
    b1 = np.zeros(128, np.float32)
    b2 = np.zeros(128, np.float32)
    r, z, n = slice(0, 64), slice(64, 128), slice(128, 192)
    lo, hi = slice(0, 64), slice(64, 128)
    zc, rc = (lo, hi) if blend_lo else (hi, lo)   # z on blend side, r opposite
    inc, hnc = (lo, hi) if blend_lo else (hi, lo)
    # W1: r and z gates
    W1[in_rows, zc] = Wih[z].T
    W1[h_rows, zc] = Whh[z].T
    W1[in_rows, rc] = Wih[r].T
    W1[h_rows, rc] = Whh[r].T
    # W2: in (x part of n-gate) on blend side, hn (h part) on r side
    W2[in_rows, inc] = Wih[n].T
    W2[h_rows, hnc] = Whh[n].T
    b1[zc] = bih[z] + bhh[z]
    b1[rc] = bih[r] + bhh[r]
    b2[inc] = bih[n]   # tanh bias (blend side)
    b2[hnc] = bhh[n]   # hn bias (r side, applied inside STT)
    return W1, W2, b1, b2


def _prep(inputs, BC):
    """Host-side packing of all weights -> dict of replicated device arrays."""
    g = lambda k: np.asarray(inputs[k])
    out = {}
    # encoder L0: stream [h0@0:64 ; x@64:70], blend LO, K=70
    W1, W2, b1, b2 = _pack_cell(g('enc0_Wih'), g('enc0_Whh'), g('enc0_bih'),
                                g('enc0_bhh'), slice(64, 70), slice(0, 64),
                                True, 70)
    out['w1_e0'], out['w2_e0'] = W1.astype(NPBF), W2.astype(NPBF)
    be0_1, be0_2 = b1, b2
    # encoder L1: stream [h0@0:64 ; h1@64:128], blend HI, K=128
    W1, W2, b1, b2 = _pack_cell(g('enc1_Wih'), g('enc1_Whh'), g('enc1_bih'),
                                g('enc1_bhh'), slice(0, 64), slice(64, 128),
                                False, 128)
    out['w1_e1'], out['w2_e1'] = W1.astype(NPBF), W2.astype(NPBF)
    be1_1, be1_2 = b1, b2
    # decoder L0: stream [h0d@0:64 ; cv@64:65], blend LO, K=65
    W1, W2, b1, b2 = _pack_cell(g('dec0_Wih'), g('dec0_Whh'), g('dec0_bih'),
                                g('dec0_bhh'), slice(64, 65), slice(0, 64),
                                True, 65)
    out['w1_d0'], out['w2_d0'] = W1.astype(NPBF), W2.astype(NPBF)
    bd0_1, bd0_2 = b1, b2
    # decoder L1: blend HI, K=128
    W1, W2, b1, b2 = _pack_cell(g('dec1_Wih'), g('dec1_Whh'), g('dec1_bih'),
                                g('dec1_bhh'), slice(0, 64), slice(64, 128),
                                False, 128)
    out['w1_d1'], out['w2_d1'] = W1.astype(NPBF), W2.astype(NPBF)
    bd1_1, bd1_2 = b1, b2
    # heads: stationary rows are h1d (stream partitions 64:128)
    won = np.zeros((64, 1), np.float32)
    won[:, 0] = g('on_w')[0]
    wcv = np.zeros((64, 1), np.float32)
    wcv[:, 0] = g('cv_w')[0]
    out['w_on'], out['w_cv'] = won.astype(NPBF), wcv.astype(NPBF)
    # bias pack (128, 10): cols 0..7 = cell biases, col 8 = -on_b @row64,
    # col 9 = cv_b @row64
    bias = np.zeros((128, 10), np.float32)
    for j, b in enumerate([be0_1, be0_2, be1_1, be1_2,
                           bd0_1, bd0_2, bd1_1, bd1_2]):
        bias[:, j] = b
    bias[64, 8] = -float(g('on_b')[0])
    bias[64, 9] = float(g('cv_b')[0])
    out['biases'] = bias
    # identity for PE accumulation (both row halves hold I64)
    ident = np.zeros((128, 64), np.float32)
    ident[0:64] = np.eye(64)
    ident[64:128] = np.eye(64)
    out['ident'] = ident.astype(NPBF)
    return out


# ------------------------------------------------------------------ device build
def _build(L, T, BC):
    M = BC // 2
    nc = bacc.Bacc("TRN2", target_bir_lowering=False, debug=False,
                   num_devices=NCORES)
    dram = {}
    for name, shape, dt in [
        ('xT', [L, NIN, BC], BF16),
        ('w1_e0', [70, 128], BF16), ('w2_e0', [70, 128], BF16),
        ('w1_e1', [128, 128], BF16), ('w2_e1', [128, 128], BF16),
        ('w1_d0', [65, 128], BF16), ('w2_d0', [65, 128], BF16),
        ('w1_d1', [128, 128], BF16), ('w2_d1', [128, 128], BF16),
        ('w_on', [64, 1], BF16), ('w_cv', [64, 1], BF16),
        ('biases', [128, 10], F32), ('ident', [128, 64], BF16),
    ]:
        dram[name] = nc.dram_tensor(name, shape, dt, kind="ExternalInput").ap()
    stg = nc.dram_tensor("stg", [T, BC], BF16, kind="ExternalOutput").ap()

    LO, HI = slice(0, 64), slice(64, 128)

    with tile.TileContext(nc) as tc:
        const = tc.alloc_tile_pool(name="const", bufs=1)
        work = tc.alloc_tile_pool(name="work", bufs=3)

        # ---- constants into SBUF
        cw = {}
        for name in ['w1_e0', 'w2_e0', 'w1_e1', 'w2_e1', 'w1_d0', 'w2_d0',
                     'w1_d1', 'w2_d1']:
            t_ = const.tile(list(dram[name].shape), BF16, name=f"c_{name}")
            nc.sync.dma_start(out=t_, in_=dram[name])
            cw[name] = t_
        whead = const.tile([128, 2], BF16, name="c_whead")
        nc.sync.dma_start(out=whead[64:128, 0:1], in_=dram['w_on'])
        nc.sync.dma_start(out=whead[64:128, 1:2], in_=dram['w_cv'])
        bias = const.tile([128, 10], F32, name="c_bias")
        nc.sync.dma_start(out=bias, in_=dram['biases'])
        ident = const.tile([128, 64], BF16, name="c_ident")
        nc.sync.dma_start(out=ident, in_=dram['ident'])

        bcol = lambda j: bias[:, j:j + 1]

        # ---- persistent stream tiles
        s0 = [const.tile([70, BC], BF16, name=f"s0_{i}") for i in range(3)]
        s1 = [const.tile([128, BC], BF16, name=f"s1_{i}") for i in range(2)]
        sd0 = [const.tile([65, BC], BF16, name=f"sd0_{i}") for i in range(2)]
        sd1 = [const.tile([128, BC], BF16, name=f"sd1_{i}") for i in range(2)]

        # init: h0 = h1 = 0; x[0] loaded
        nc.vector.memset(s0[0][LO, :], 0.0)
        nc.vector.memset(s1[0][HI, :], 0.0)
        nc.sync.dma_start(out=s0[0][64:70, :], in_=dram['xT'][0])

        def halves(ap):
            return (ap[:, 0:M], ap[:, M:2 * M])

        def hs(ap, h):
            return ap[:, h * M:(h + 1) * M]

        def cell_alloc(pool, tag, ptag):
            """Per-(cell, tick) tiles shared by the two batch-half chains."""
            p1 = pool.tile([128, BC], F32, name=f"p1_{tag}", tag=f"{ptag}p1")
            p2 = pool.tile([128, BC], F32, name=f"p2_{tag}", tag=f"{ptag}p2")
            rz = work.tile([128, BC], BF16, name=f"rz_{tag}", tag=f"rz{ptag}")
            rhn = work.tile([128, BC], BF16, name=f"rhn_{tag}",
                            tag=f"rhn{ptag}")
            return p1, p2, rz, rhn

        # Stage emitters. Engines execute their instruction streams in
        # (scheduler-chosen, roughly program) order, so ops are emitted
        # stage-by-stage with the two batch halves interleaved — half B's
        # stage-k op sits right behind half A's in each engine queue and the
        # two chains pipeline instead of running back to back.
        def st_mm1(ct, S, h, w1):
            nc.tensor.matmul(hs(ct[0], h), w1, hs(S, h), start=True, stop=True)

        def st_mm2(ct, S, h, w2):
            nc.tensor.matmul(hs(ct[1], h), w2, hs(S, h), start=True, stop=True)

        def st_sig(ct, h, b1c):
            nc.scalar.activation(out=hs(ct[2], h), in_=hs(ct[0], h),
                                 func=ACTF.Sigmoid, bias=b1c, scale=1.0)

        def st_rhn(ct, h, b2c, blend_lo):
            rs = HI if blend_lo else LO
            nc.vector.scalar_tensor_tensor(
                out=hs(ct[3], h)[rs, :], in0=hs(ct[1], h)[rs, :],
                scalar=b2c[rs, :], in1=hs(ct[2], h)[rs, :],
                op0=ALU.add, op1=ALU.mult)

        def st_acc(ct, h, blend_lo):
            bl, rs = (LO, HI) if blend_lo else (HI, LO)
            tp_acc = (64, 0) if blend_lo else (0, 64)
            nc.tensor.matmul(hs(ct[1], h)[bl, :], ident[rs, :],
                             hs(ct[3], h)[rs, :], start=False, stop=True,
                             tile_position=tp_acc)

        def st_tanh(ct, h, b2c, blend_lo, n_t):
            bl = LO if blend_lo else HI
            nc.scalar.activation(out=hs(n_t, h)[bl, :], in_=hs(ct[1], h)[bl, :],
                                 func=ACTF.Tanh, bias=b2c[bl, :], scale=1.0)

        def st_u(uv, S_old, h, n_t, part):
            nc.vector.tensor_tensor(out=hs(uv[0], h)[part, :],
                                    in0=hs(S_old, h)[part, :],
                                    in1=hs(n_t, h)[part, :], op=ALU.subtract)

        def st_v(uv, rz, h, side):
            nc.vector.tensor_tensor(out=hs(uv[1], h)[side, :],
                                    in0=hs(rz, h)[side, :],
                                    in1=hs(uv[0], h)[side, :], op=ALU.mult)

        def st_add(uv, h, n_t, part, out_ap):
            nc.vector.tensor_tensor(out=out_ap, in0=hs(n_t, h)[part, :],
                                    in1=hs(uv[1], h)[part, :], op=ALU.add)

        # ================= encoder (layer-staggered) =================
        # Tick k: L0 consumes x_k and h0_{k-1}; L1 consumes [h0_{k-1};
        # h1_{k-2}] (= S1 tile). The layer chains are independent (join only
        # at the shared blend) and the batch halves A/B form two independent
        # chains that interleave on every engine.
        eps = tc.alloc_tile_pool(name="eps", bufs=1, space="PSUM")
        nc.vector.memset(s1[0][LO, :], 0.0)
        nc.vector.memset(s1[1][HI, :], 0.0)
        for k in range(L + 1):
            S0, S0n = s0[k % 3], s0[(k + 1) % 3]
            S1, S1n = s1[k % 2], s1[(k + 1) % 2]
            if 1 <= k + 1 < L:
                nc.sync.dma_start(out=S0n[64:70, :], in_=dram['xT'][k + 1])
            n_t = work.tile([128, BC], BF16, name=f"n_{k}", tag="n")
            uv = (work.tile([128, BC], BF16, name=f"u_{k}", tag="u"),
                  work.tile([128, BC], BF16, name=f"v_{k}", tag="v"))
            has0, has1 = k < L, k >= 1
            if has0 and has1:
                part = slice(0, 128)
            else:
                part = LO if has0 else HI
            cells = []
            if has0:
                ct0 = cell_alloc(eps, f"e0_{k}", "l0")
                cells.append((ct0, S0, cw['w1_e0'], cw['w2_e0'], bcol(0),
                              bcol(1), True))
            if has1:
                ct1 = cell_alloc(eps, f"e1_{k}", "l1")
                cells.append((ct1, S1, cw['w1_e1'], cw['w2_e1'], bcol(2),
                              bcol(3), False))
            for h in range(2):
                for c in cells:
                    st_mm1(c[0], c[1], h, c[2])
            for h in range(2):
                for c in cells:
                    st_mm2(c[0], c[1], h, c[3])
            for h in range(2):
                for c in cells:
                    st_sig(c[0], h, c[4])
            for h in range(2):
                for c in cells:
                    st_rhn(c[0], h, c[5], c[6])
            for h in range(2):
                for c in cells:
                    st_acc(c[0], h, c[6])
            for h in range(2):
                for c in cells:
                    st_tanh(c[0], h, c[5], c[6], n_t)
            for h in range(2):
                st_u(uv, S1, h, n_t, part)
            for h in range(2):
                if has0:
                    st_v(uv, ct0[2], h, LO)
                if has1:
                    st_v(uv, ct1[2], h, HI)
            for h in range(2):
                st_add(uv, h, n_t, part, hs(S1n, h)[part, :])
            if k < L - 1:
                for h in range(2):
                    nc.vector.tensor_copy(hs(S0n, h)[LO, :],
                                          hs(S1n, h)[LO, :])

        # ================= transition =================
        # decoder h0 init = h0_{L-1} (in s1[L%2][LO]); h1 init = h1_{L-1}
        # (in s1[(L+1)%2][HI]); cv init = 0
        nc.vector.tensor_copy(sd0[0][LO, :], s1[L % 2][LO, :])
        nc.vector.tensor_copy(sd1[0][HI, :], s1[(L + 1) % 2][HI, :])
        nc.vector.memset(sd0[0][64:65, :], 0.0)
        eps.release()

        # ================= decoder =================
        dps = tc.alloc_tile_pool(name="dps", bufs=1, space="PSUM")
        for t in range(T):
            D0, D0n = sd0[t % 2], sd0[(t + 1) % 2]
            D1, D1n = sd1[t % 2], sd1[(t + 1) % 2]
            n_t = work.tile([128, BC], BF16, name=f"nd_{t}", tag="n")
            uv = (work.tile([128, BC], BF16, name=f"ud_{t}", tag="u"),
                  work.tile([128, BC], BF16, name=f"vd_{t}", tag="v"))
            cvsb = work.tile([66, BC], BF16, name=f"cvsb_{t}", tag="cvsb")
            ct0 = cell_alloc(dps, f"d0_{t}", "d0")
            ct1 = cell_alloc(dps, f"d1_{t}", "d1")
            # heads psum shares the d0 tags: their lifetimes dovetail with the
            # real cv -> next-step-dec0 dependency (8 psum banks total)
            pon = dps.tile([65, BC], F32, name=f"pon_{t}", tag="d0p1")
            pcv = dps.tile([65, BC], F32, name=f"pcv_{t}", tag="d0p2")
            c0 = (ct0, D0, cw['w1_d0'], cw['w2_d0'], bcol(4), bcol(5), True)
            c1 = (ct1, D1, cw['w1_d1'], cw['w2_d1'], bcol(6), bcol(7), False)
            # dec0 (both halves, stage-interleaved)
            for h in range(2):
                st_mm1(ct0, D0, h, c0[2])
                st_mm2(ct0, D0, h, c0[3])
            for h in range(2):
                st_sig(ct0, h, c0[4])
            for h in range(2):
                st_rhn(ct0, h, c0[5], True)
            for h in range(2):
                st_acc(ct0, h, True)
            for h in range(2):
                st_tanh(ct0, h, c0[5], True, n_t)
            for h in range(2):
                st_u(uv, D0, h, n_t, LO)
            for h in range(2):
                st_v(uv, ct0[2], h, LO)
            for h in range(2):
                st_add(uv, h, n_t, LO, hs(D1, h)[LO, :])
            for h in range(2):
                nc.vector.tensor_copy(hs(D0n, h)[LO, :], hs(D1, h)[LO, :])
            # dec1
            for h in range(2):
                st_mm1(ct1, D1, h, c1[2])
                st_mm2(ct1, D1, h, c1[3])
            for h in range(2):
                st_sig(ct1, h, c1[4])
            for h in range(2):
                st_rhn(ct1, h, c1[5], False)
            for h in range(2):
                st_acc(ct1, h, False)
            for h in range(2):
                st_tanh(ct1, h, c1[5], False, n_t)
            for h in range(2):
                st_u(uv, D1, h, n_t, HI)
            for h in range(2):
                st_v(uv, ct1[2], h, HI)
            for h in range(2):
                st_add(uv, h, n_t, HI, hs(D1n, h)[HI, :])
            # heads on h1' (= D1n[HI]); psum tiles live at partition 64
            for h in range(2):
                nc.tensor.matmul(hs(pon, h)[64:65, :], whead[64:128, 0:1],
                                 hs(D1n, h)[HI, :], start=True, stop=True,
                                 tile_position=(64, 64))
                nc.tensor.matmul(hs(pcv, h)[64:65, :], whead[64:128, 1:2],
                                 hs(D1n, h)[HI, :], start=True, stop=True,
                                 tile_position=(64, 64))
            for h in range(2):
                nc.vector.tensor_scalar_add(out=hs(cvsb, h)[64:65, :],
                                            in0=hs(pcv, h)[64:65, :],
                                            scalar1=bias[64:65, 9:10])
            for h in range(2):
                nc.vector.scalar_tensor_tensor(
                    out=hs(D0n, h)[64:65, :], in0=hs(pon, h)[64:65, :],
                    scalar=bias[64:65, 8:9], in1=hs(cvsb, h)[64:65, :],
                    op0=ALU.is_gt, op1=ALU.mult)
            nc.gpsimd.dma_start(out=stg[t:t + 1, :], in_=D0n[64:65, :])

        dps.release()
        work.release()
        const.release()

    nc.compile()
    return nc


def _get_nc(L, T, BC):
    key = (L, T, BC)
    if key not in _BUILD_CACHE:
        _BUILD_CACHE[key] = _build(L, T, BC)
    return _BUILD_CACHE[key]


# ------------------------------------------------------------------ entry point
def kernel(**inputs):
    x = np.asarray(inputs['x'])
    B, L, _ = x.shape
    T = T_OUT
    BC = B // NCORES
    nc = _get_nc(L, T, BC)

    packed = _prep(inputs, BC)
    in_maps = []
    for c in range(NCORES):
        xs = x[c * BC:(c + 1) * BC].astype(np.float32)      # (BC, L, 6)
        xT = np.ascontiguousarray(xs.transpose(1, 2, 0)).astype(NPBF)
        m = dict(packed)
        m['xT'] = xT
        in_maps.append(m)

    res = run_bass_kernel_spmd(nc, in_maps, core_ids=list(range(NCORES)))
    out = np.empty((B, T, 1), np.float32)
    for c in range(NCORES):
        stg = np.asarray(res.results[c]['stg'], np.float32)  # (T, BC)
        out[c * BC:(c + 1) * BC, :, 0] = stg.T
    return out
